# revision 1
# baseline (speedup 1.0000x reference)
"""BitBottleneck (ternary-quantized 3x3 conv x2 + BN + SiLU + residual) on 8 trn2 cores.

Strategy:
  - Data-parallel over batch: 32 images -> 4 per core, no collectives.
  - Ternary quantization is folded on host: w_q = s*t with t in {-1,0,+1};
    conv(x, s*t) == s*conv(x, t), and s folds into the BN scale. The weights
    the PE sees are exactly representable in fp16, so matmul precision is
    limited only by the fp16 rounding of activations (~1e-4 end-to-end).
  - The residual uses a hi/lo split (x == fp16(x) + xlo) so the shortcut
    term is fp32-exact while matmuls read only the fp16 hi plane.
  - Each conv is an implicit GEMM: per 8-row output block (N=448) accumulate
    9 taps x cin-blocks of [K=128, M=128] x [K=128, N=448] matmuls into one
    PSUM bank. Inputs live in SBUF zero-padded to 58x58 so every tap is a
    pure strided view. fp16 streams 1 col/cycle at 2.4GHz (fp32 is 4x
    slower; 4-byte f32r caps at ~2.13GHz on the moving-operand read path).
  - ScalarE applies BN+SiLU (per-partition scale/bias) reading PSUM directly;
    VectorE adds the residual; DMA queues are split (weights on scalar,
    x-image chunks alternating sync/gpsimd, stores on gpsimd) so the first
    matmuls start as soon as the first image rows and W1 land.
"""

import sys

if "/opt/trn_rl_repo" not in sys.path:
    sys.path.insert(0, "/opt/trn_rl_repo")

import numpy as np

B, C, H, W = 32, 256, 56, 56
HID = 128
NCORES = 8
IPC = B // NCORES  # images per core
HP, WP = H + 2, W + 2  # padded 58x58
PADN = HP * WP  # 3364
RB = 8  # output rows per row-block
NRB = H // RB  # 7
NMM = RB * W  # 448 matmul free dim
BN_EPS = 1e-5
Q_EPS = 1e-5

import os

MM_DT = os.environ.get("KMMDT", "f16")  # "f32r", "bf16", or "f16"

_CACHE = {}


def _build_nc():
    if "nc" in _CACHE:
        return _CACHE["nc"]

    import concourse.bass as bass
    import concourse.mybir as mybir
    import concourse.tile as tile
    from concourse import bacc

    f32 = mybir.dt.float32
    f32r = mybir.dt.float32r
    # dtype for matmul-feeding tensors. All of these stream 1 col/cycle on
    # the PE (plain fp32 runs at 1/4 rate), but 4-byte f32r caps at ~2.13GHz
    # effective on the moving-operand read path while 2-byte dtypes hit the
    # full 2.4GHz. f16 keeps 10 mantissa bits vs bf16's 8.
    mmdt = {"f32r": f32r, "bf16": mybir.dt.bfloat16, "f16": mybir.dt.float16}[MM_DT]
    two_byte = MM_DT in ("bf16", "f16")
    SILU = mybir.ActivationFunctionType.Silu

    nc = bacc.Bacc("TRN2", target_bir_lowering=False, debug=False)

    xp = nc.dram_tensor("xp", [IPC, 2, 128, PADN], mmdt, kind="ExternalInput")
    w1t = nc.dram_tensor("w1t", [128, 18 * 128], mmdt, kind="ExternalInput")
    w2t = nc.dram_tensor("w2t", [128, 18 * 128], mmdt, kind="ExternalInput")
    ab1 = nc.dram_tensor("ab1", [128, 2], f32, kind="ExternalInput")
    ab2 = nc.dram_tensor("ab2", [128, 4], f32, kind="ExternalInput")
    if two_byte:
        # residual correction plane: xlo = x - mmdt(x), so the shortcut adds
        # hi + lo = x to ~fp32 precision while matmuls read only hi.
        xlo = nc.dram_tensor("xlo", [IPC, 2, 128, H * W], mmdt, kind="ExternalInput")
    y = nc.dram_tensor("y", [IPC, 2, 128, H * W], f32, kind="ExternalOutput")

    TAPS = [(ky, kx) for ky in range(3) for kx in range(3)]

    with tile.TileContext(nc) as tc:
        with (
            tc.tile_pool(name="consts", bufs=1) as cpool,
            tc.tile_pool(name="xin", bufs=3) as xpool,
            tc.tile_pool(name="hbuf", bufs=1) as hpool,
            tc.tile_pool(name="stage", bufs=4) as spool,
            tc.tile_pool(name="outs", bufs=4) as opool,
            tc.tile_pool(name="ps", bufs=4, space=bass.MemorySpace.PSUM) as pspool,
        ):
            # weights/consts on the scalar DGE queue so the x-image loads
            # (sync/gpsimd queues) aren't stuck behind them.
            W1 = cpool.tile([128, 18 * 128], mmdt, tag="W1")
            nc.scalar.dma_start(W1[:], w1t[:, :])
            AB1 = cpool.tile([128, 2], f32, tag="AB1")
            nc.scalar.dma_start(AB1[:], ab1[:, :])
            AB2 = cpool.tile([128, 4], f32, tag="AB2")
            nc.scalar.dma_start(AB2[:], ab2[:, :])
            W2 = cpool.tile([128, 18 * 128], mmdt, tag="W2")
            nc.scalar.dma_start(W2[:], w2t[:, :])

            # h ping-pong tiles; zero the padding border once, interior is
            # fully rewritten every image. Writes go through ScalarE's Silu
            # with scale=0 (silu(0)=0): works for every matmul dtype incl.
            # f32r, and avoids a 4-byte-packet DMA storm.
            zrow = cpool.tile([128, WP], f32, tag="zrow")
            nc.gpsimd.memset(zrow[:], 0.0)
            hts = []
            for j in range(2):
                ht = hpool.tile([128, PADN], mmdt, tag=f"h{j}")
                hv = ht[:].rearrange("p (r c) -> p r c", r=HP, c=WP)
                for dst, n in (
                    (hv[:, 0, :], WP),
                    (hv[:, HP - 1, :], WP),
                    (hv[:, 1 : HP - 1, 0:1], HP - 2),
                    (hv[:, 1 : HP - 1, WP - 1 : WP], HP - 2),
                ):
                    nc.scalar.activation(dst, zrow[:, :n], SILU, bias=0.0, scale=0.0)
                hts.append(ht)

            # PE warm-up: junk matmuls during the ~9us DMA queue-arming dead
            # time, so the HAM clock gate reaches K=8/8 (2.4GHz) before the
            # first real matmul instead of running the first ~3.4us at 1.2GHz.
            junk = cpool.tile([128, NMM], mybir.dt.float16, tag="junk")
            nc.gpsimd.memset(junk[:], 0.0)
            for _ in range(12):
                pw = pspool.tile([128, NMM], f32, tag="ps1")
                nc.tensor.matmul(pw[:], junk[:, :128], junk[:], start=True, stop=True)

            # padded-row chunks: rowblock r needs padded rows [8r, 8r+10)
            # first chunk is small so the first matmuls can start early
            XCHUNKS = [(0, 10), (10, 18), (18, 34), (34, 50), (50, HP)]

            for img in range(IPC):
                X = xpool.tile([128, 2 * PADN], mmdt, tag="X")
                for ci, (r0, r1) in enumerate(XCHUNKS):
                    for blk in range(2):
                        # chunk 0 rides the earliest-armed queue entirely so
                        # the first PSUM group isn't gated on a later queue
                        if ci == 0:
                            eng = nc.sync
                        else:
                            eng = nc.sync if (ci * 2 + blk) % 2 == 0 else nc.gpsimd
                        eng.dma_start(
                            X[:, blk * PADN + r0 * WP : blk * PADN + r1 * WP],
                            xp[img, blk, :, r0 * WP : r1 * WP],
                        )
                Xv = X[:].rearrange("p (b r c) -> p b r c", b=2, r=HP, c=WP)
                if two_byte:
                    XL = xpool.tile([128, 2 * H * W], mmdt, tag="XL")
                    for blk in range(2):
                        nc.gpsimd.dma_start(
                            XL[:, blk * H * W : (blk + 1) * H * W], xlo[img, blk, :, :]
                        )
                    XLv = XL[:].rearrange("p (b n) -> p b n", b=2)
                ht = hts[img % 2]
                hv = ht[:].rearrange("p (r c) -> p r c", r=HP, c=WP)

                # conv1: 256 -> 128, BN+SiLU into padded h interior
                for r in range(NRB):
                    ps1 = pspool.tile([128, NMM], f32, tag="ps1")
                    k = 0
                    for t, (ky, kx) in enumerate(TAPS):
                        for blk in range(2):
                            rhs = Xv[:, blk, RB * r + ky : RB * r + ky + RB, kx : kx + W]
                            lhsT = W1[:, (t * 2 + blk) * 128 : (t * 2 + blk + 1) * 128]
                            nc.tensor.matmul(
                                ps1[:],
                                lhsT,
                                rhs,
                                start=(k == 0),
                                stop=(k == 17),
                            )
                            k += 1
                    nc.scalar.activation(
                        hv[:, 1 + RB * r : 1 + RB * r + RB, 1 : 1 + W],
                        ps1[:],
                        SILU,
                        bias=AB1[:, 1:2],
                        scale=AB1[:, 0:1],
                    )

                # conv2: 128 -> 256 (two cout blocks), BN+SiLU, +residual, store
                for r in range(NRB):
                    for cb in range(2):
                        ps2 = pspool.tile([128, NMM], f32, tag="ps2")
                        for t, (ky, kx) in enumerate(TAPS):
                            rhs = hv[:, RB * r + ky : RB * r + ky + RB, kx : kx + W]
                            lhsT = W2[:, (t * 2 + cb) * 128 : (t * 2 + cb + 1) * 128]
                            nc.tensor.matmul(
                                ps2[:],
                                lhsT,
                                rhs,
                                start=(t == 0),
                                stop=(t == 8),
                            )
                        st = spool.tile([128, NMM], f32, tag="st")
                        nc.scalar.activation(
                            st[:],
                            ps2[:],
                            SILU,
                            bias=AB2[:, 2 * cb + 1 : 2 * cb + 2],
                            scale=AB2[:, 2 * cb : 2 * cb + 1],
                        )
                        ot = opool.tile([128, NMM], f32, tag="ot")
                        xres = Xv[:, cb, 1 + RB * r : 1 + RB * r + RB, 1 : 1 + W]
                        if MM_DT == "f32r":
                            xres = xres.bitcast(f32)
                        nc.vector.tensor_add(ot[:], st[:], xres)
                        if two_byte:
                            nc.vector.tensor_add(
                                ot[:], ot[:], XLv[:, cb, r * NMM : (r + 1) * NMM]
                            )
                        # stores ride gpsimd; for the last image the other
                        # queues are idle, so spread stores to shorten the
                        # final flush the exit drain waits on
                        if img == IPC - 1:
                            seng = (nc.gpsimd, nc.sync, nc.scalar)[(r * 2 + cb) % 3]
                        else:
                            seng = nc.gpsimd
                        seng.dma_start(
                            y[img, cb, :, r * NMM : (r + 1) * NMM], ot[:]
                        )

    nc.compile()
    _CACHE["nc"] = nc
    return nc


def _quant_ternary(w):
    """Match jnp: s = max(median(|w|), Q_EPS); t = clip(round(w/s), -1, 1)."""
    w = np.asarray(w, np.float32)
    s = np.float32(np.median(np.abs(w)))
    s = np.maximum(s, np.float32(Q_EPS))
    t = np.clip(np.round(w / s), np.float32(-1.0), np.float32(1.0)).astype(np.float32)
    return s, t


def prepare_inputs(x, w1, g1, b1, m1, v1, w2, g2, b2, m2, v2):
    """Host-side prep: quantize+fold weights, pad x, build per-core in_maps."""
    x = np.asarray(x, np.float32)

    s1, t1 = _quant_ternary(w1)
    s2, t2 = _quant_ternary(w2)

    inv1 = np.asarray(g1, np.float32) / np.sqrt(np.asarray(v1, np.float32) + np.float32(BN_EPS))
    a1 = (s1 * inv1).astype(np.float32)  # [HID]
    c1 = (np.asarray(b1, np.float32) - np.asarray(m1, np.float32) * inv1).astype(np.float32)
    inv2 = np.asarray(g2, np.float32) / np.sqrt(np.asarray(v2, np.float32) + np.float32(BN_EPS))
    a2 = (s2 * inv2).astype(np.float32)  # [C]
    c2 = (np.asarray(b2, np.float32) - np.asarray(m2, np.float32) * inv2).astype(np.float32)

    ab1 = np.stack([a1, c1], axis=1).astype(np.float32)  # [128, 2]
    a2b = a2.reshape(2, 128)
    c2b = c2.reshape(2, 128)
    ab2 = np.stack([a2b[0], c2b[0], a2b[1], c2b[1]], axis=1).astype(np.float32)  # [128,4]

    # lhsT layouts: [cin128, ((ky*3+kx)*2 + blk)*128 + cout]
    w1t = (
        t1.reshape(HID, 2, 128, 3, 3).transpose(2, 3, 4, 1, 0).reshape(128, 18 * 128)
    ).astype(np.float32).copy()
    w2t = (
        t2.reshape(2, 128, HID, 3, 3).transpose(2, 3, 4, 0, 1).reshape(128, 18 * 128)
    ).astype(np.float32).copy()

    if MM_DT == "bf16":
        import ml_dtypes

        mmnp = np.dtype(ml_dtypes.bfloat16)
    elif MM_DT == "f16":
        mmnp = np.dtype(np.float16)
    else:
        mmnp = np.dtype(np.float32)
    two_byte = MM_DT in ("bf16", "f16")
    w1t = w1t.astype(mmnp)
    w2t = w2t.astype(mmnp)

    xhi = x.astype(mmnp)
    xpad = np.zeros((B, C, HP, WP), mmnp)
    xpad[:, :, 1 : 1 + H, 1 : 1 + W] = xhi
    xp = xpad.reshape(NCORES, IPC, 2, 128, PADN)
    if two_byte:
        xlo = (x - xhi.astype(np.float32)).astype(mmnp)
        xlo = xlo.reshape(NCORES, IPC, 2, 128, H * W)

    in_maps = []
    for c in range(NCORES):
        m = {
            "xp": np.ascontiguousarray(xp[c]),
            "w1t": w1t,
            "w2t": w2t,
            "ab1": ab1,
            "ab2": ab2,
        }
        if two_byte:
            m["xlo"] = np.ascontiguousarray(xlo[c])
        in_maps.append(m)
    return in_maps


def assemble_output(per_core_results):
    ys = np.stack([r["y"] for r in per_core_results])  # [8, IPC, 2, 128, H*W]
    return ys.reshape(B, C, H, W).astype(np.float32)


def run_spmd(in_maps, **kwargs):
    from concourse.bass_utils import run_bass_kernel_spmd

    nc = _build_nc()
    return run_bass_kernel_spmd(nc, in_maps, core_ids=list(range(NCORES)), **kwargs)


def kernel(**inputs):
    in_maps = prepare_inputs(**inputs)
    res = run_spmd(in_maps)
    return assemble_output(res.results)



# revision 4
# speedup vs baseline: 1.6699x; 1.6699x over previous
"""BitBottleneck (ternary-quantized 3x3 conv x2 + BN + SiLU + residual) on 8 trn2 cores.

Strategy (v2, fp8 DoubleRow):
  - Data-parallel over batch: 32 images -> 4 per core, no collectives.
  - Ternary weights {-1,0,+1} are exact in fp8 e4m3; the per-tensor scale s
    folds into the BN scale. Activations are quantized to e4m3 (x on host,
    h on-chip via ScalarE's RNE cast on the SiLU write). Measured end-to-end
    rel err 1.89e-2 (budget 2e-2); the conv arithmetic itself is exact.
  - PE runs fp8 DoubleRow (2 MACs/cell/cycle): conv1 pairs the two cin-128
    blocks (contraction 256 per stream); conv2 pairs taps — 3 horizontal
    pairs (stride 1), 1 vertical pair (stride 64), 1 normal tap — 4 DR + 1
    plain matmul per output block instead of 9.
  - Weight-stationary schedule: for each weight tile, stream all 7 row-block
    PSUM banks (one image's conv output) before switching tiles, so each
    LDWEIGHTS (DoubleRow disables fast-weight-load) is amortized over 7
    matmuls and hides behind streaming.
  - x/h planes are padded to row pitch 64 so every tap is a strided view and
    DR pair strides meet the %16 LDWEIGHTS constraint where it applies.
  - Residual uses a separate fp16(x) plane (2.1e-4 error, negligible here);
    outputs are stored fp16 and upcast on host.
"""

import sys

if "/opt/trn_rl_repo" not in sys.path:
    sys.path.insert(0, "/opt/trn_rl_repo")

import numpy as np

B, C, H, W = 32, 256, 56, 56
HID = 128
NCORES = 8
IPC = B // NCORES  # images per core
PITCH = 64  # padded row pitch (58 cols used)
HP = 58  # padded rows
PLANE = HP * PITCH  # 3712 bytes/partition per cin block (fp8)
RB = 8  # output rows per row-block
NRB = H // RB  # 7
NMM = RB * W  # 448 matmul free dim
BN_EPS = 1e-5
Q_EPS = 1e-5

_CACHE = {}


def _build_nc():
    if "nc" in _CACHE:
        return _CACHE["nc"]

    import concourse.bass as bass
    import concourse.mybir as mybir
    import concourse.tile as tile
    from concourse import bacc
    from concourse.bass import AP

    f32 = mybir.dt.float32
    f16 = mybir.dt.float16
    f8 = mybir.dt.float8e4
    SILU = mybir.ActivationFunctionType.Silu
    DR = mybir.MatmulPerfMode.DoubleRow

    nc = bacc.Bacc("TRN2", target_bir_lowering=False, debug=False)

    xp8 = nc.dram_tensor("xp8", [IPC, 2, 128, PLANE], f8, kind="ExternalInput")
    xr = nc.dram_tensor("xr", [IPC, 2, 128, H * W], f16, kind="ExternalInput")
    w1t = nc.dram_tensor("w1t", [128, 9 * 256], f8, kind="ExternalInput")
    w2t = nc.dram_tensor("w2t", [128, 2 * 1152], f8, kind="ExternalInput")
    ab1 = nc.dram_tensor("ab1", [128, 2], f32, kind="ExternalInput")
    ab2 = nc.dram_tensor("ab2", [128, 4], f32, kind="ExternalInput")
    y = nc.dram_tensor("y", [IPC, 2, 128, H * W], f16, kind="ExternalOutput")

    def pv(base, off, dims):
        """Strided (possibly overlapping) free view: dims = [(stride, n), ...]."""
        return AP(base.tensor, base.offset + off, [list(base.ap[0])] + [[s, n] for s, n in dims])

    # conv2 tap plan per cout block: (kind, ap-offset-fn, pair-stride)
    #   3 horizontal DR pairs (ky,0)+(ky,1), 1 vertical DR pair (0,2)+(1,2),
    #   1 normal tap (2,2)
    C2PLAN = [
        ("dr", lambda rb: (8 * rb + 0) * PITCH + 0, 1),
        ("dr", lambda rb: (8 * rb + 1) * PITCH + 0, 1),
        ("dr", lambda rb: (8 * rb + 2) * PITCH + 0, 1),
        ("dr", lambda rb: (8 * rb + 0) * PITCH + 2, PITCH),
        ("n", lambda rb: (8 * rb + 2) * PITCH + 2, None),
    ]

    with tile.TileContext(nc) as tc:
        with (
            tc.tile_pool(name="consts", bufs=1) as cpool,
            tc.tile_pool(name="xin", bufs=4) as xpool,
            tc.tile_pool(name="hbuf", bufs=1) as hpool,
            tc.tile_pool(name="outs", bufs=6) as opool,
            tc.tile_pool(name="ps", bufs=8, space=bass.MemorySpace.PSUM) as pspool,
        ):
            # weights/consts on the scalar DGE queue so the x-image loads
            # (sync/gpsimd queues) aren't stuck behind them.
            W1 = cpool.tile([128, 9 * 256], f8, tag="W1")
            nc.scalar.dma_start(W1[:], w1t[:, :])
            AB1 = cpool.tile([128, 2], f32, tag="AB1")
            nc.scalar.dma_start(AB1[:], ab1[:, :])
            AB2 = cpool.tile([128, 4], f32, tag="AB2")
            nc.scalar.dma_start(AB2[:], ab2[:, :])
            W2 = cpool.tile([128, 2 * 1152], f8, tag="W2")
            nc.scalar.dma_start(W2[:], w2t[:, :])

            # h ping-pong tiles; zero the whole plane once (border rows/cols
            # are never rewritten; interior is). ScalarE Silu with scale=0
            # writes exact fp8 zeros.
            zrow = cpool.tile([128, PITCH * 2], f32, tag="zrow")
            nc.gpsimd.memset(zrow[:], 0.0)
            hts = []
            for j in range(2):
                ht = hpool.tile([128, PLANE], f8, tag=f"h{j}")
                hv = ht[:].rearrange("p (r c) -> p r c", r=HP, c=PITCH)
                for dst, n in (
                    (hv[:, 0, :], PITCH),
                    (hv[:, HP - 1, :], PITCH),
                    (hv[:, 1 : HP - 1, 0:1], HP - 2),
                    (hv[:, 1 : HP - 1, 57:58], HP - 2),
                ):
                    nc.scalar.activation(dst, zrow[:, :n], SILU, bias=0.0, scale=0.0)
                hts.append(ht)

            # PE warm-up: junk matmuls during the ~9us DMA queue-arming dead
            # time, so the HAM clock gate reaches K=8/8 (2.4GHz) before the
            # first real matmul instead of running the first ~3.4us at 1.2GHz.
            junk = cpool.tile([128, NMM], mybir.dt.float16, tag="junk")
            nc.gpsimd.memset(junk[:], 0.0)
            for _ in range(14):
                pw = pspool.tile([128, NMM], f32, tag="ps")
                nc.tensor.matmul(pw[:], junk[:, :128], junk[:], start=True, stop=True)

            for img in range(IPC):
                X8 = xpool.tile([128, 2 * PLANE], f8, tag="X8")
                XR = xpool.tile([128, 2 * H * W], f16, tag="XR")
                if img == 0:
                    # first image: chunk rows across both queues so conv1 can
                    # start as soon as the full plane lands
                    XCH = [(0, 15), (15, 30), (30, 44), (44, HP)]
                    for ci, (r0, r1) in enumerate(XCH):
                        for blk in range(2):
                            eng = nc.sync if (ci * 2 + blk) % 2 == 0 else nc.gpsimd
                            eng.dma_start(
                                X8[:, blk * PLANE + r0 * PITCH : blk * PLANE + r1 * PITCH],
                                xp8[img, blk, :, r0 * PITCH : r1 * PITCH],
                            )
                    for blk in range(2):
                        eng = nc.sync if blk == 0 else nc.gpsimd
                        eng.dma_start(
                            XR[:, blk * H * W : (blk + 1) * H * W], xr[img, blk, :, :]
                        )
                else:
                    for blk in range(2):
                        eng = nc.sync if blk == 0 else nc.gpsimd
                        eng.dma_start(
                            X8[:, blk * PLANE : (blk + 1) * PLANE], xp8[img, blk, :, :]
                        )
                        eng.dma_start(
                            XR[:, blk * H * W : (blk + 1) * H * W], xr[img, blk, :, :]
                        )

                X8b = X8[:]
                XRv = XR[:].rearrange("p (b r c) -> p b r c", b=2, r=H, c=W)
                ht = hts[img % 2]
                hb = ht[:]
                hv = ht[:].rearrange("p (r c) -> p r c", r=HP, c=PITCH)

                # conv1: 256 -> 128. Weight-stationary: tap-outer over the 7
                # row-block PSUM banks; DR pairs the two cin blocks.
                ps1 = [pspool.tile([128, NMM], f32, tag="ps", name=f"ps1_{img}_{rb}") for rb in range(NRB)]
                for t in range(9):
                    ky, kx = t // 3, t % 3
                    wt = W1[:, t * 256 : (t + 1) * 256].rearrange("p (i m) -> p i m", i=2)
                    for rb in range(NRB):
                        rhs = pv(X8b, (8 * rb + ky) * PITCH + kx, [(PLANE, 2), (PITCH, 8), (1, 56)])
                        nc.tensor.matmul(
                            ps1[rb][:], wt, rhs, start=(t == 0), stop=(t == 8), perf_mode=DR
                        )
                # BN+SiLU into padded h interior (fp8 RNE write)
                for rb in range(NRB):
                    nc.scalar.activation(
                        hv[:, 1 + RB * rb : 1 + RB * rb + RB, 1:57],
                        ps1[rb][:],
                        SILU,
                        bias=AB1[:, 1:2],
                        scale=AB1[:, 0:1],
                    )

                # conv2: 128 -> 256 (two cout blocks), weight-stationary
                for cb in range(2):
                    ps2 = [pspool.tile([128, NMM], f32, tag="ps", name=f"ps2_{img}_{cb}_{rb}") for rb in range(NRB)]
                    for k, (kind, offn, pstride) in enumerate(C2PLAN):
                        base = cb * 1152 + k * 256
                        if kind == "dr":
                            wt = W2[:, base : base + 256].rearrange("p (i m) -> p i m", i=2)
                        else:
                            wt = W2[:, base : base + 128]
                        for rb in range(NRB):
                            if kind == "dr":
                                rhs = pv(hb, offn(rb), [(pstride, 2), (PITCH, 8), (1, 56)])
                                nc.tensor.matmul(
                                    ps2[rb][:], wt, rhs,
                                    start=(k == 0), stop=(k == 4), perf_mode=DR,
                                )
                            else:
                                rhs = pv(hb, offn(rb), [(PITCH, 8), (1, 56)])
                                nc.tensor.matmul(
                                    ps2[rb][:], wt, rhs, start=(k == 0), stop=(k == 4)
                                )
                    for rb in range(NRB):
                        st = opool.tile([128, NMM], f16, tag="st")
                        nc.scalar.activation(
                            st[:],
                            ps2[rb][:],
                            SILU,
                            bias=AB2[:, 2 * cb + 1 : 2 * cb + 2],
                            scale=AB2[:, 2 * cb : 2 * cb + 1],
                        )
                        ot = opool.tile([128, NMM], f16, tag="ot")
                        nc.vector.tensor_add(
                            ot[:], st[:], XRv[:, cb, RB * rb : RB * rb + RB, :]
                        )
                        # stores ride gpsimd; for the last image the other
                        # queues are idle, so spread stores to shorten the
                        # final flush the exit drain waits on
                        if img == IPC - 1:
                            seng = (nc.gpsimd, nc.sync, nc.scalar)[(rb * 2 + cb) % 3]
                        else:
                            seng = nc.gpsimd
                        seng.dma_start(y[img, cb, :, rb * NMM : (rb + 1) * NMM], ot[:])

    nc.compile()
    _CACHE["nc"] = nc
    return nc


def _quant_ternary(w):
    """Match jnp: s = max(median(|w|), Q_EPS); t = clip(round(w/s), -1, 1)."""
    w = np.asarray(w, np.float32)
    s = np.float32(np.median(np.abs(w)))
    s = np.maximum(s, np.float32(Q_EPS))
    t = np.clip(np.round(w / s), np.float32(-1.0), np.float32(1.0)).astype(np.float32)
    return s, t


def prepare_inputs(x, w1, g1, b1, m1, v1, w2, g2, b2, m2, v2):
    """Host-side prep: quantize+fold weights, pad/cast x, build per-core in_maps."""
    import ml_dtypes

    E4M3 = np.dtype(ml_dtypes.float8_e4m3fn)
    F16 = np.dtype(np.float16)

    x = np.asarray(x, np.float32)

    s1, t1 = _quant_ternary(w1)
    s2, t2 = _quant_ternary(w2)

    inv1 = np.asarray(g1, np.float32) / np.sqrt(np.asarray(v1, np.float32) + np.float32(BN_EPS))
    a1 = (s1 * inv1).astype(np.float32)
    c1 = (np.asarray(b1, np.float32) - np.asarray(m1, np.float32) * inv1).astype(np.float32)
    inv2 = np.asarray(g2, np.float32) / np.sqrt(np.asarray(v2, np.float32) + np.float32(BN_EPS))
    a2 = (s2 * inv2).astype(np.float32)
    c2 = (np.asarray(b2, np.float32) - np.asarray(m2, np.float32) * inv2).astype(np.float32)

    ab1 = np.stack([a1, c1], axis=1).astype(np.float32)  # [128, 2]
    a2b = a2.reshape(2, 128)
    c2b = c2.reshape(2, 128)
    ab2 = np.stack([a2b[0], c2b[0], a2b[1], c2b[1]], axis=1).astype(np.float32)  # [128,4]

    # conv1 DR weight tiles: [cin_p, tap, pair(blk), cout]
    # value = t1[cout, blk*128 + cin_p, ky, kx]
    w1t = (
        t1.reshape(HID, 2, 128, 3, 3)  # [m, i, p, ky, kx]
        .transpose(2, 3, 4, 1, 0)      # [p, ky, kx, i, m]
        .reshape(128, 9 * 256)
    ).astype(E4M3)

    # conv2 tiles per cout block: 3 horizontal DR pairs (ky,0)+(ky,1),
    # 1 vertical DR pair (0,2)+(1,2), 1 normal (2,2)
    t2b = t2.reshape(2, 128, 128, 3, 3)  # [cb, m, p, ky, kx]
    w2arr = np.zeros((128, 2, 1152), np.float32)  # [p, cb, cols]
    for cb in range(2):
        col = 0
        for ky in range(3):  # horizontal pairs
            for i, kx in enumerate((0, 1)):
                w2arr[:, cb, col : col + 128] = t2b[cb, :, :, ky, kx].T
                col += 128
        for i, (ky, kx) in enumerate(((0, 2), (1, 2))):  # vertical pair
            w2arr[:, cb, col : col + 128] = t2b[cb, :, :, ky, kx].T
            col += 128
        w2arr[:, cb, col : col + 128] = t2b[cb, :, :, 2, 2].T
    w2t = w2arr.reshape(128, 2 * 1152).astype(E4M3)

    # x planes: fp8 padded (pitch 64) for matmuls, fp16 unpadded for residual
    xpad = np.zeros((B, C, HP, PITCH), E4M3)
    xpad[:, :, 1 : 1 + H, 1 : 1 + W] = x.astype(E4M3)
    xp8 = xpad.reshape(NCORES, IPC, 2, 128, PLANE)
    xr = x.astype(F16).reshape(NCORES, IPC, 2, 128, H * W)

    in_maps = []
    for c in range(NCORES):
        in_maps.append(
            {
                "xp8": np.ascontiguousarray(xp8[c]),
                "xr": np.ascontiguousarray(xr[c]),
                "w1t": w1t,
                "w2t": w2t,
                "ab1": ab1,
                "ab2": ab2,
            }
        )
    return in_maps


def assemble_output(per_core_results):
    ys = np.stack([r["y"] for r in per_core_results])  # [8, IPC, 2, 128, H*W]
    return ys.astype(np.float32).reshape(B, C, H, W)


def run_spmd(in_maps, **kwargs):
    from concourse.bass_utils import run_bass_kernel_spmd

    nc = _build_nc()
    return run_bass_kernel_spmd(nc, in_maps, core_ids=list(range(NCORES)), **kwargs)


def kernel(**inputs):
    in_maps = prepare_inputs(**inputs)
    res = run_spmd(in_maps)
    return assemble_output(res.results)
